# revision 18
# baseline (speedup 1.0000x reference)
"""v3: fp16 compute path with SBUF-resident state (no fp32 DRAM master).

State X lives in SBUF as fp16 for all 10 steps; the update X += delta is
applied in fp16 (rounding drift ~1e-3 abs over 10 steps, well inside the
2e-2 gate). This removes the per-step fp32 master RMW (DRAM round-trip,
Pool adds, ACT refresh casts) of v2.

Per step, per (k=c-half, a=component):
  s-term  fs(j) = X(s_j+1)-X(s_j-1) over S_LOC+2 planes, *= dcf  (DVE 2x)
          dlt = fs[2:] - fs[:-2]                                 (DVE 2x)
  r-term  P = X(r+2)-X(r); P *= dcs-view; w = P[2:]-P[:-2];
          edge-column fixups on w; dlt += w                      (DVE 2x)
  c-term  gc = D1 @ X (PE/PSUM); gcb = cast (ACT); fc = gcb*dcf
          (Pool or DVE); dcps = D2 @ fc (PE/PSUM); dpb = cast (ACT);
          dlt += dpb                                             (DVE)
  update  X += dlt (Pool or DVE, per-ka knob)
dcs(s,r) = dcp(s,r+1) is a flat shifted view of dcf — no separate tile.
Halo exchange over cores: AllGather of 2-plane boundary bounces (fp16).
"""
import sys
sys.path.insert(0, '/opt/trn_rl_repo')
import numpy as np
import concourse.bass as bass
import concourse.mybir as mybir
from concourse import tile, bacc

F32 = mybir.dt.float32
FP16 = mybir.dt.float16
I32 = mybir.dt.int32

# Offline TimelineSim can't model collectives; when True the AllGather is
# replaced with equivalent-traffic local DMAs so the module stays single-core.
SIM_MODE = False

# engine knobs, tuned against TimelineSim: for each (k,a) flat index 0..5,
# which engine runs fc = gcb*dcf and the state update X += dlt.
FC_POOL = (False, False, False, False, False, False)
XU_POOL = (True, True, True, True, True, True)
XU_PSUM = False  # direct-PSUM update serializes PE on the PSUM bank; keep the cast


class Cfg:
    def __init__(self, NC=8, S_LOC=24, A=3, R=192, C=192, B=8, NT=10, DT=0.01):
        self.NC, self.S_LOC, self.A, self.R, self.C = NC, S_LOC, A, R, C
        self.B, self.NT, self.DT = B, NT, DT
        assert S_LOC % B == 0
        self.NB = S_LOC // B
        self.W = C // 2
        self.P_IN = self.W + 2
        self.P_G = self.W + 1
        self.S_E = S_LOC + 4
        self.S = NC * S_LOC
        self.cmap = [
            list(range(self.W)) + [self.W, self.W + 1],
            list(range(self.W, 2 * self.W)) + [self.W - 1, self.W - 2],
        ]
        self.fmap = [m[: self.W + 1] for m in self.cmap]


def grad_coeff(n, i_out, i_in):
    if i_out == 0:
        return {0: -1.0, 1: 1.0}.get(i_in, 0.0)
    if i_out == n - 1:
        return {n - 1: 1.0, n - 2: -1.0}.get(i_in, 0.0)
    return {i_out + 1: 0.5, i_out - 1: -0.5}.get(i_in, 0.0)


def build_dmats(cfg):
    C = cfg.C
    d1s, d2s = [], []
    for k in range(2):
        cmap, fmap = cfg.cmap[k], cfg.fmap[k]
        own = range(cfg.W * k, cfg.W * (k + 1))
        D1 = np.zeros((cfg.P_IN, cfg.P_G), np.float32)
        for q, cq in enumerate(fmap):
            for p, cp in enumerate(cmap):
                D1[p, q] = 2.0 * grad_coeff(C, cq, cp)
        D2 = np.zeros((cfg.P_G, cfg.W), np.float32)
        for m, cm in enumerate(own):
            for q, cq in enumerate(fmap):
                D2[q, m] = 2.0 * grad_coeff(C, cm, cq)
        d1s.append(D1)
        d2s.append(D2)
    return d1s, d2s


def build(cfg):
    NC, A, R, W, P_IN, P_G = cfg.NC, cfg.A, cfg.R, cfg.W, cfg.P_IN, cfg.P_G
    S_LOC, S_E, B, NB, NT = cfg.S_LOC, cfg.S_E, cfg.B, cfg.NB, cfg.NT
    FD = S_LOC * R
    FSP = (S_LOC + 2) * R          # fs free size (S_LOC+2 planes)

    nc = bacc.Bacc("TRN2", target_bir_lowering=False)

    xb_ext = [nc.dram_tensor(f"xb{k}", [P_IN, A, S_E, R], FP16, kind="ExternalInput")
              for k in range(2)]
    dcf_ext = [nc.dram_tensor(f"dcf{k}", [P_IN, S_LOC + 2, R], FP16, kind="ExternalInput")
               for k in range(2)]
    dce_ext = [nc.dram_tensor(f"dce{k}", [P_IN, S_LOC, 6], F32, kind="ExternalInput")
               for k in range(2)]
    d1_ext = [nc.dram_tensor(f"d1m{k}", [P_IN, P_G], FP16, kind="ExternalInput")
              for k in range(2)]
    d2_ext = [nc.dram_tensor(f"d2m{k}", [P_G, W], FP16, kind="ExternalInput")
              for k in range(2)]
    ip_ext = nc.dram_tensor("ipm", [W, W], FP16, kind="ExternalInput")
    im_ext = nc.dram_tensor("imm", [W, W], FP16, kind="ExternalInput")
    scal_ext = nc.dram_tensor("scal", [P_IN, 8], F32, kind="ExternalInput")
    offs_ext = nc.dram_tensor("offs", [1, 2], I32, kind="ExternalInput")
    out_ext = nc.dram_tensor("out", [2 * W, A, S_LOC, R], F32, kind="ExternalOutput")

    with tile.TileContext(nc) as tc:
        with (
            tc.tile_pool(name="res", bufs=1) as res,
            tc.tile_pool(name="fs_p", bufs=2) as fs_p,
            tc.tile_pool(name="fc_p", bufs=2) as fc_p,
            tc.tile_pool(name="pp_p", bufs=2) as pp_p,
            tc.tile_pool(name="w_p", bufs=1) as w_p,
            tc.tile_pool(name="st_p", bufs=2) as st_p,   # epilogue f32 staging
            tc.tile_pool(name="sm_p", bufs=2) as sm_p,
            tc.tile_pool(name="ps_g", bufs=1, space="PSUM") as ps_g,
            tc.tile_pool(name="ps_d", bufs=1, space="PSUM") as ps_d,
            tc.tile_pool(name="dram", bufs=1, space="DRAM") as dram,
        ):
            xhl = [[res.tile([P_IN, 2, R], FP16, name=f"xhl{k}{a}") for a in range(A)]
                   for k in range(2)]
            xm_t = [[res.tile([P_IN, FD + 2], FP16, name=f"xm{k}{a}") for a in range(A)]
                    for k in range(2)]
            xhr = [[res.tile([P_IN, 2, R], FP16, name=f"xhr{k}{a}") for a in range(A)]
                   for k in range(2)]
            dcf_t = [res.tile([P_IN, S_LOC + 2, R], FP16, name=f"dcft{k}") for k in range(2)]
            dce_t = [res.tile([P_IN, S_LOC, 6], F32, name=f"dcet{k}") for k in range(2)]
            d1t = [res.tile([P_IN, P_G], FP16, name=f"d1t{k}") for k in range(2)]
            d2t = [res.tile([P_G, W], FP16, name=f"d2t{k}") for k in range(2)]
            ipt = res.tile([W, W], FP16, name="ipt")
            imt = res.tile([W, W], FP16, name="imt")
            scal_t = res.tile([P_IN, 8], F32, name="scal_t")

            bounce = [dram.tile([4, 2 * W, R], FP16, name=f"bounce{a}")
                      for a in range(A)]

            # ---- prologue ----
            for k in range(2):
                for a in range(A):
                    nc.sync.dma_start(xhl[k][a][:], xb_ext[k][:, a, 0:2, :])
                    nc.sync.dma_start(
                        xm_t[k][a][0:P_IN, 0:FD],
                        xb_ext[k][:, a, 2:2 + S_LOC, :].rearrange(
                            "p s r -> p (s r)"))
                    nc.vector.memset(xm_t[k][a][0:P_IN, FD:FD + 2], 0.0)
                    nc.sync.dma_start(xhr[k][a][:], xb_ext[k][:, a, S_E - 2:S_E, :])
                nc.sync.dma_start(dcf_t[k][:], dcf_ext[k][:])
                nc.sync.dma_start(dce_t[k][:], dce_ext[k][:])
                nc.sync.dma_start(d1t[k][:], d1_ext[k][:])
                nc.sync.dma_start(d2t[k][:], d2_ext[k][:])
            nc.sync.dma_start(ipt[:], ip_ext[:])
            nc.sync.dma_start(imt[:], im_ext[:])
            nc.sync.dma_start(scal_t[:], scal_ext[:])

            lreg = nc.scalar.register("lreg").__enter__()
            rreg = nc.scalar.register("rreg").__enter__()
            nc.scalar.reg_load(lreg, offs_ext[0:1, 0:1])
            nc.scalar.reg_load(rreg, offs_ext[0:1, 1:2])

            V = nc.vector
            SC = nc.scalar
            GP = nc.gpsimd
            TT = mybir.AluOpType

            for t_step in range(NT):
                # ---- ghost blends on halo planes ----
                for k in range(2):
                    for a in range(A):
                        M3 = xm_t[k][a][0:P_IN, 0:FD].rearrange(
                            "p (s r) -> p s r", r=R)
                        for (gidx, gdst, g0, g1) in (
                            (0, xhl[k][a][0:W, 1, :], M3[0:W, 0, :], M3[0:W, 1, :]),
                            (4, xhr[k][a][0:W, 0, :], M3[0:W, S_LOC - 1, :],
                             M3[0:W, S_LOC - 2, :]),
                        ):
                            a1 = sm_p.tile([W, R], FP16, tag="gsa1")
                            a2 = sm_p.tile([W, R], FP16, tag="gsa2")
                            V.tensor_scalar_mul(a1[:], g1,
                                                scal_t[0:W, gidx + 2:gidx + 3])
                            V.scalar_tensor_tensor(a2[:], g0,
                                                   scal_t[0:W, gidx + 1:gidx + 2],
                                                   a1[:], TT.mult, TT.add)
                            V.scalar_tensor_tensor(gdst, gdst,
                                                   scal_t[0:W, gidx:gidx + 1],
                                                   a2[:], TT.mult, TT.add)

                if t_step < NT - 1:
                    loff = nc.scalar.snap(lreg)
                    roff = nc.scalar.snap(rreg)
                for a in range(A):
                    for k in range(2):
                        dcf_f = dcf_t[k].rearrange("p s r -> p (s r)")
                        ka = k * A + a
                        Mf = xm_t[k][a]
                        M3 = Mf[0:P_IN, 0:FD].rearrange("p (s r) -> p s r", r=R)
                        HL, HR = xhl[k][a], xhr[k][a]

                        # ---- s-flux over S_LOC+2 planes (full ka) ----
                        fs = fs_p.tile([W, FSP], FP16, tag="fs")
                        fsv = fs.rearrange("p (s r) -> p s r", r=R)
                        V.tensor_tensor(fs[0:W, 2 * R:S_LOC * R],
                                        Mf[0:W, 2 * R:S_LOC * R],
                                        Mf[0:W, 0:(S_LOC - 2) * R], TT.subtract)
                        V.tensor_tensor(fsv[0:W, 0, :], M3[0:W, 0, :],
                                        HL[0:W, 0, :], TT.subtract)
                        V.tensor_tensor(fsv[0:W, 1, :], M3[0:W, 1, :],
                                        HL[0:W, 1, :], TT.subtract)
                        V.tensor_tensor(fsv[0:W, S_LOC, :], HR[0:W, 0, :],
                                        M3[0:W, S_LOC - 2, :], TT.subtract)
                        V.tensor_tensor(fsv[0:W, S_LOC + 1, :], HR[0:W, 1, :],
                                        M3[0:W, S_LOC - 1, :], TT.subtract)
                        V.tensor_tensor(fs[:], fs[:], dcf_f[0:W, 0:FSP], TT.mult)
                        # global s-edge one-sided blends (masked by scal)
                        b1 = sm_p.tile([W, R], FP16, tag="fga1")
                        b2 = sm_p.tile([W, R], FP16, tag="fga2")
                        V.tensor_scalar_mul(b1[:], fsv[0:W, 2, :], scal_t[0:W, 2:3])
                        V.scalar_tensor_tensor(b2[:], fsv[0:W, 1, :],
                                               scal_t[0:W, 1:2], b1[:],
                                               TT.mult, TT.add)
                        V.scalar_tensor_tensor(fsv[0:W, 0, :], fsv[0:W, 0, :],
                                               scal_t[0:W, 0:1], b2[:],
                                               TT.mult, TT.add)
                        e = S_LOC + 1
                        b3 = sm_p.tile([W, R], FP16, tag="fga1")
                        b4 = sm_p.tile([W, R], FP16, tag="fga2")
                        V.tensor_scalar_mul(b3[:], fsv[0:W, e - 2, :], scal_t[0:W, 6:7])
                        V.scalar_tensor_tensor(b4[:], fsv[0:W, e - 1, :],
                                               scal_t[0:W, 5:6], b3[:],
                                               TT.mult, TT.add)
                        V.scalar_tensor_tensor(fsv[0:W, e, :], fsv[0:W, e, :],
                                               scal_t[0:W, 4:5], b4[:],
                                               TT.mult, TT.add)

                        # ---- r-term, even-shift P scheme ----
                        P = pp_p.tile([W, FD + 4], FP16, tag="pp")
                        V.tensor_tensor(P[0:W, 2:2 + FD],
                                        Mf[0:W, 2:FD + 2],
                                        Mf[0:W, 0:FD], TT.subtract)
                        # dcs(s,r) = dcp(s,r+1) == flat dcf view shifted R+1
                        V.tensor_tensor(P[0:W, 2:2 + FD], P[0:W, 2:2 + FD],
                                        dcf_f[0:W, R + 1:R + 1 + FD], TT.mult)
                        V.memset(P[0:W, 0:2], 0.0)
                        V.memset(P[0:W, FD + 2:FD + 4], 0.0)
                        w = w_p.tile([W, FD], FP16, tag="w")
                        V.tensor_tensor(w[:], P[0:W, 2:2 + FD], P[0:W, 0:FD],
                                        TT.subtract)
                        # ---- r edge fixup columns on w ----
                        wv = w.rearrange("p (s r) -> p s r", r=R)
                        t1 = sm_p.tile([W, S_LOC, 2], F32, tag="te1")
                        t2 = sm_p.tile([W, S_LOC, 2], F32, tag="te2")
                        t3 = sm_p.tile([W, S_LOC, 2], F32, tag="te3")
                        V.tensor_tensor(t1[:], M3[0:W, :, 1:R:R - 2],
                                        M3[0:W, :, 0:R - 1:R - 2], TT.subtract)
                        V.tensor_tensor(t2[:], M3[0:W, :, 2:R:R - 3],
                                        M3[0:W, :, 0:R - 2:R - 3], TT.subtract)
                        V.tensor_tensor(t3[:], M3[0:W, :, 3:R - 1:R - 5],
                                        M3[0:W, :, 1:R - 3:R - 5], TT.subtract)
                        Bt = sm_p.tile([W, S_LOC, 2], F32, tag="teB")
                        At = sm_p.tile([W, S_LOC, 2], F32, tag="teA")
                        Ct = sm_p.tile([W, S_LOC, 2], F32, tag="teC")
                        V.tensor_tensor(Bt[:], t1[:], dce_t[k][0:W, :, 0:2], TT.mult)
                        V.tensor_tensor(At[:], t2[:], dce_t[k][0:W, :, 2:4], TT.mult)
                        V.tensor_tensor(Ct[:], t3[:], dce_t[k][0:W, :, 4:6], TT.mult)
                        V.scalar_tensor_tensor(wv[0:W, :, 0:R:R - 1], At[:], 2.0,
                                               Bt[:], TT.mult, TT.subtract)
                        V.scalar_tensor_tensor(wv[0:W, :, 1:R - 1:R - 3], Bt[:], -0.5,
                                               Ct[:], TT.mult, TT.add)

                        # ---- per block: c-term matmuls + PSUM-accumulated
                        # s-diff and r-term, then fp16 state update ----
                        for b in range(NB):
                            p0 = b * B
                            FDB = B * R
                            base = p0 * R
                            gc = ps_g.tile([P_G, FDB], F32, tag="gc")
                            for q0 in range(0, FDB, 512):
                                q1 = min(q0 + 512, FDB)
                                nc.tensor.matmul(gc[:, q0:q1], d1t[k][:],
                                                 Mf[0:P_IN, base + q0:base + q1],
                                                 start=True, stop=True)
                            gcb = fc_p.tile([P_G, FDB], FP16, tag="gcb")
                            SC.copy(gcb[:], gc[:, :])
                            fc = fc_p.tile([P_G, FDB], FP16, tag="fc")
                            eng = GP if FC_POOL[ka] else V
                            eng.tensor_tensor(
                                fc[0:P_G, :], gcb[0:P_G, :],
                                dcf_f[0:P_G, base + R:base + R + FDB], TT.mult)
                            dcps = ps_d.tile([W, FDB], F32, tag="dcps")
                            for q0 in range(0, FDB, 512):
                                q1 = min(q0 + 512, FDB)
                                nc.tensor.matmul(dcps[:, q0:q1], d2t[k][:],
                                                 fc[0:P_G, q0:q1],
                                                 start=True, stop=False)
                                nc.tensor.matmul(dcps[:, q0:q1], ipt[:],
                                                 w[0:W, base + q0:base + q1],
                                                 start=False, stop=False)
                                nc.tensor.matmul(dcps[:, q0:q1], ipt[:],
                                                 fs[0:W, 2 * R + base + q0:
                                                    2 * R + base + q1],
                                                 start=False, stop=False)
                                nc.tensor.matmul(dcps[:, q0:q1], imt[:],
                                                 fs[0:W, base + q0:base + q1],
                                                 start=False, stop=True)
                            xeng = GP if XU_POOL[ka] else V
                            if XU_PSUM:
                                xeng.tensor_tensor(Mf[0:W, base:base + FDB],
                                                   Mf[0:W, base:base + FDB],
                                                   dcps[:, :], TT.add)
                            else:
                                dpb = fc_p.tile([W, FDB], FP16, tag="dpb")
                                SC.copy(dpb[:], dcps[:, :])
                                xeng.tensor_tensor(Mf[0:W, base:base + FDB],
                                                   Mf[0:W, base:base + FDB],
                                                   dpb[:], TT.add)

                        # stage this half's boundary planes for the exchange
                        if t_step < NT - 1:
                            M3b = Mf[0:W, 0:FD].rearrange("p (s r) -> p s r", r=R)
                            nc.sync.dma_start(
                                bounce[a][0:2, k * W:(k + 1) * W, :]
                                .transpose([1, 0, 2]), M3b[0:W, 0:2, :])
                            nc.sync.dma_start(
                                bounce[a][2:4, k * W:(k + 1) * W, :]
                                .transpose([1, 0, 2]),
                                M3b[0:W, S_LOC - 2:S_LOC, :])

                    # ---- per-component halo exchange, overlapped with the
                    # remaining components' compute ----
                    if t_step < NT - 1:
                        gathered = dram.tile(
                            [NC, 4, 2 * W, R], FP16,
                            name=f"gathered{t_step}_{a}",
                            addr_space="Local" if SIM_MODE else
                            ("Shared" if NC > 4 else "Local"))
                        if SIM_MODE:
                            for ci in range(NC):
                                nc.sync.dma_start(gathered[ci], bounce[a][:])
                        else:
                            nc.gpsimd.collective_compute(
                                "AllGather", TT.bypass,
                                replica_groups=[list(range(NC))],
                                ins=[bounce[a].opt()], outs=[gathered.opt()])
                        for k in range(2):
                            c0, c1 = k * W, (k + 1) * W
                            nc.scalar.dma_start(
                                xhl[k][a][0:W, :, :],
                                gathered[bass.ds(loff, 1), 2:4, c0:c1, :]
                                .transpose([0, 2, 1, 3]))
                            nc.scalar.dma_start(
                                xhr[k][a][0:W, :, :],
                                gathered[bass.ds(roff, 1), 0:2, c0:c1, :]
                                .transpose([0, 2, 1, 3]))
                        # c-halo refresh on owned planes
                        nc.sync.dma_start(
                            xm_t[0][a][W:W + 2, 0:FD],
                            xm_t[1][a][0:2, 0:FD])
                        nc.sync.dma_start(
                            xm_t[1][a][W:W + 1, 0:FD],
                            xm_t[0][a][W - 1:W, 0:FD])
                        nc.sync.dma_start(
                            xm_t[1][a][W + 1:W + 2, 0:FD],
                            xm_t[0][a][W - 2:W - 1, 0:FD])

            # ---- epilogue: cast fp16 state -> f32 out ----
            for k in range(2):
                for a in range(A):
                    Mf = xm_t[k][a]
                    of = out_ext[k * W:(k + 1) * W, a, :, :].rearrange(
                        "c s r -> c (s r)")
                    HFD = FD // 2
                    for h in range(2):
                        stg = st_p.tile([W, HFD], F32, tag="stg")
                        SC.copy(stg[:], Mf[0:W, h * HFD:(h + 1) * HFD])
                        nc.sync.dma_start(of[0:W, h * HFD:(h + 1) * HFD], stg[:])
    nc.finalize()
    return nc


def prep_inputs(cfg, X_full, dc_full):
    d1s, d2s = build_dmats(cfg)
    S_LOC, S_E, A, R, W = cfg.S_LOC, cfg.S_E, cfg.A, cfg.R, cfg.W
    dcp = (0.25 * cfg.DT * dc_full).astype(np.float32)   # [S,R,C]
    in_maps = []
    for i in range(cfg.NC):
        s_idx = (np.arange(i * S_LOC - 2, i * S_LOC + S_LOC + 2)) % cfg.S
        so = s_idx[2:S_E - 2]
        m = {}
        for k in range(2):
            cm = np.array(cfg.cmap[k])
            xk = X_full[s_idx][:, :, cm, :]            # [S_E, R, P_IN, A]
            m[f"xb{k}"] = np.ascontiguousarray(
                xk.transpose(2, 3, 0, 1)).astype(np.float16)
            dk = dcp[s_idx[1:S_E - 1]][:, :, cm]       # [S_LOC+2, R, P_IN]
            m[f"dcf{k}"] = np.ascontiguousarray(
                dk.transpose(2, 0, 1)).astype(np.float16)
            # dce cols: [4dc'(0), -4dc'(R-1), dc'(1), -dc'(R-2), dc'(2), -dc'(R-3)]
            d0 = dcp[so][:, :, cm]                     # [S_LOC, R, P_IN]
            de = np.stack([
                4.0 * d0[:, 0, :], -4.0 * d0[:, R - 1, :],
                d0[:, 1, :], -d0[:, R - 2, :],
                d0[:, 2, :], -d0[:, R - 3, :],
            ], axis=-1)                                # [S_LOC, P_IN, 6]
            m[f"dce{k}"] = np.ascontiguousarray(de.transpose(1, 0, 2)).astype(np.float32)
            m[f"d1m{k}"] = d1s[k].astype(np.float16)
            m[f"d2m{k}"] = d2s[k].astype(np.float16)
        m["ipm"] = np.eye(cfg.W, dtype=np.float16)
        m["imm"] = (-np.eye(cfg.W)).astype(np.float16)
        gl = 1.0 if i == 0 else 0.0
        gr = 1.0 if i == cfg.NC - 1 else 0.0
        sc = np.array([1 - gl, 2 * gl, -gl, gl, 1 - gr, 2 * gr, -gr, gr], np.float32)
        m["scal"] = np.broadcast_to(sc, (cfg.P_IN, 8)).copy()
        m["offs"] = np.array([[(i - 1) % cfg.NC, (i + 1) % cfg.NC]], np.int32)
        in_maps.append(m)
    return in_maps


_BUILT_CACHE = {}


def kernel(X, diff_coeff, nt):
    """Full inputs in, full output out. X: [192,192,192,3] f32,
    diff_coeff: [192,192,192] f32, nt: int."""
    X = np.asarray(X, dtype=np.float32)
    dc = np.asarray(diff_coeff, dtype=np.float32)
    nt = int(nt)
    if nt <= 0:
        return X.copy()

    cfg = Cfg(NC=8, S_LOC=X.shape[0] // 8, A=X.shape[3], R=X.shape[1],
              C=X.shape[2], B=8, NT=nt, DT=0.01)
    key = (cfg.NC, cfg.S_LOC, cfg.A, cfg.R, cfg.C, cfg.B, nt)
    if key not in _BUILT_CACHE:
        _BUILT_CACHE[key] = build(cfg)
    nc = _BUILT_CACHE[key]

    in_maps = prep_inputs(cfg, X, dc)
    from concourse.bass_utils import run_bass_kernel_spmd
    res = run_bass_kernel_spmd(nc, in_maps, list(range(cfg.NC)), trace=False)
    outs = [r["out"].transpose(2, 3, 0, 1) for r in res.results]
    return np.ascontiguousarray(np.concatenate(outs, axis=0))
